# revision 3
# baseline (speedup 1.0000x reference)
"""Two-layer GAT on 8 Trainium2 NeuronCores (Bass/Tile) — v3.

Destination-major edge layout: edge slot (p = dst rank within chunk,
t = in-edge index). Chunks are 128 dsts of similar degree (degree-sorted
permutation per core), so the per-chunk tile count J[c] tracks the chunk's
max degree with little padding.

  * No one-hot masks: the scatter is J[c] identity-lhsT matmuls that
    accumulate [128, SCW] in PSUM (a per-partition reduce over t).
  * alpha_dst is per-partition: one narrow indirect gather per chunk (L1),
    a direct column read (L2); broadcast along t at 2x.
  * Pad slots point at a sentinel table row (alpha_src = -80 -> ex ~ 1e-7,
    zero payload) so they vanish from numerator and denominator.
  * Gathers are per-tile [P, 1]-offset indirect DMAs (the only shape the
    SWDGE ucode supports).
bf16 data path; sharded Phase A + AllGather of the projected tables.
"""
import os
import sys
import types

for _p in ("/opt/trn_rl_repo", "/root/.axon_site/_ro/trn_rl_repo"):
    if os.path.isdir(_p) and _p not in sys.path:
        sys.path.insert(0, _p)

import numpy as np
import ml_dtypes


def _ensure_axon_hooks_shim():
    try:
        import antenv
    except ImportError:
        return
    if "antenv.axon_hooks" in sys.modules:
        return
    try:
        import antenv.axon_hooks  # noqa: F401
        return
    except ImportError:
        pass
    m = types.ModuleType("antenv.axon_hooks")
    m._hook = None
    m.set_axon_ntff_profile_hook = lambda h: setattr(m, "_hook", h)
    m.get_axon_ntff_profile_hook = lambda: m._hook
    sys.modules["antenv.axon_hooks"] = m
    antenv.axon_hooks = m


_ensure_axon_hooks_shim()

import concourse.bacc as bacc
import concourse.bass as bass
import concourse.mybir as mybir
import concourse.tile as tile
from concourse import bass_utils
from concourse.masks import make_identity

# ---------------------------------------------------------------- problem dims
N = 50000
E = 800000
F_IN = 256
HEADS = 8
HID = 32
D1 = HEADS * HID          # 256
NCLS = 40
NEG = 0.2
NC = 8
NSH = N // NC             # dst nodes per core
P = 128
T1COL = D1 + HEADS        # 264 = xw | alpha_src
W1COL = D1 + 2 * HEADS    # 272 = xw | as | ad
W2COL = 44                # hw2(40) | as2 | ad2 | one | pad
SENT_AS = -80.0
EPS = 1e-20

f32 = mybir.dt.float32
bf16 = mybir.dt.bfloat16
i32 = mybir.dt.int32
AF = mybir.ActivationFunctionType
OP = mybir.AluOpType

_last_bench = None
_prog_cache = {}

bfloat16 = ml_dtypes.bfloat16


def _nchunk():
    return (NSH + P - 1) // P


# ---------------------------------------------------------------- host prep
def _prep(x, edge_index, W1, a1_src, a1_dst, b1, W2, a2_src, a2_dst, b2):
    NCH = _nchunk()
    SLOT1 = NSH + 1
    SLOT2 = NCH * P + 1
    src = np.concatenate([edge_index[0], np.arange(N, dtype=np.int32)])
    dst = np.concatenate([edge_index[1], np.arange(N, dtype=np.int32)])
    order = np.argsort(dst, kind="stable")
    src = src[order].astype(np.int64)
    dst = dst[order].astype(np.int64)
    bounds = np.searchsorted(dst, np.arange(0, N + 1, NSH))

    perms = np.empty((NC, NSH), np.int64)
    rankof = np.empty((NC, NSH), np.int64)
    degs = np.empty((NC, NSH), np.int64)
    for k in range(NC):
        lo, hi = bounds[k], bounds[k + 1]
        deg = np.bincount(dst[lo:hi] - k * NSH, minlength=NSH)
        perm = np.argsort(-deg, kind="stable")
        perms[k] = perm
        rankof[k][perm] = np.arange(NSH)
        degs[k] = deg

    J = np.ones(NCH, np.int64)
    for k in range(NC):
        dsort = degs[k][perms[k]]
        for c in range(NCH):
            seg = dsort[c * P:(c + 1) * P]
            if len(seg):
                J[c] = max(J[c], int(seg.max()))
    TCOLS = int(J.sum())
    colbase = np.concatenate([[0], np.cumsum(J)]).astype(np.int64)

    srcidx1 = np.full((NC, P, TCOLS), NSH, np.int32)        # pad -> sentinel
    srcidx2 = np.full((NC, P, TCOLS), NCH * P, np.int32)
    adidx = np.zeros((NC, P, NCH), np.int32)

    for k in range(NC):
        lo, hi = bounds[k], bounds[k + 1]
        dl = dst[lo:hi] - k * NSH
        ss = src[lo:hi]
        rank = rankof[k][dl]
        c = rank // P
        p = rank % P
        idxs = np.arange(hi - lo)
        runstart = np.r_[True, dl[1:] != dl[:-1]]
        startpos = np.maximum.accumulate(np.where(runstart, idxs, 0))
        t = idxs - startpos
        assert (t < J[c]).all()
        col = colbase[c] + t
        sk = ss // NSH
        sl = ss % NSH
        srcidx1[k, p, col] = sk * SLOT1 + sl
        srcidx2[k, p, col] = sk * SLOT2 + rankof[sk, sl]
        grid = np.minimum(np.arange(NCH)[None, :] * P
                          + np.arange(P)[:, None], NSH - 1)
        adidx[k] = perms[k][grid]

    A1s = np.zeros((D1, HEADS), np.float32)
    A1d = np.zeros((D1, HEADS), np.float32)
    for h in range(HEADS):
        A1s[h * HID:(h + 1) * HID, h] = a1_src[h]
        A1d[h * HID:(h + 1) * HID, h] = a1_dst[h]
    W1aug = np.concatenate([W1, W1 @ A1s, W1 @ A1d], axis=1)
    W2aug = np.zeros((D1, W2COL), np.float32)
    W2aug[:, :NCLS] = W2
    W2aug[:, NCLS] = (W2 @ a2_src.T)[:, 0]
    W2aug[:, NCLS + 1] = (W2 @ a2_dst.T)[:, 0]
    onecol = np.zeros((1, W2COL), np.float32)
    onecol[0, NCLS + 2] = 1.0
    sent1 = np.zeros((1, T1COL), np.float32)
    sent1[0, D1:] = SENT_AS
    sent2 = np.zeros((1, W2COL), np.float32)
    sent2[0, NCLS] = SENT_AS

    xT = np.ascontiguousarray(x.T).astype(bfloat16)

    shared = {
        "W1aug": W1aug.astype(bfloat16),
        "W2aug": W2aug.astype(bfloat16),
        "b1": b1.reshape(1, D1).astype(bfloat16),
        "onecol": onecol.astype(bfloat16),
        "b2": b2.reshape(1, NCLS).astype(bfloat16),
        "sent1": sent1.astype(bfloat16),
        "sent2": sent2.astype(bfloat16),
    }
    in_maps = []
    for k in range(NC):
        mdl = dict(shared)
        mdl["xTs"] = np.ascontiguousarray(xT[:, k * NSH:(k + 1) * NSH])
        mdl["srcidx1"] = srcidx1[k]
        mdl["srcidx2"] = srcidx2[k]
        mdl["adidx"] = adidx[k]
        in_maps.append(mdl)
    return in_maps, tuple(int(j) for j in J), perms


# ---------------------------------------------------------------- program
def _build(J):
    NCH = _nchunk()
    SLOT1 = NSH + 1
    SLOT2 = NCH * P + 1
    TCOLS = int(sum(J))
    nc = bacc.Bacc("TRN2", target_bir_lowering=False, debug=False,
                   num_devices=NC)
    g = {}

    def ein(name, shape, dt):
        g[name] = nc.dram_tensor(name, shape, dt, kind="ExternalInput").ap()

    ein("xTs", [F_IN, NSH], bf16)
    ein("W1aug", [F_IN, W1COL], bf16)
    ein("W2aug", [D1, W2COL], bf16)
    ein("b1", [1, D1], bf16)
    ein("onecol", [1, W2COL], bf16)
    ein("b2", [1, NCLS], bf16)
    ein("sent1", [1, T1COL], bf16)
    ein("sent2", [1, W2COL], bf16)
    ein("srcidx1", [P, TCOLS], i32)
    ein("srcidx2", [P, TCOLS], i32)
    ein("adidx", [P, NCH], i32)
    g["out2"] = nc.dram_tensor("out2", [NCH * P, NCLS], f32,
                               kind="ExternalOutput").ap()
    g["t1shard"] = nc.dram_tensor("t1shard", [SLOT1, T1COL], bf16,
                                  kind="Internal").ap()
    g["adshard"] = nc.dram_tensor("adshard", [NSH, HEADS], bf16,
                                  kind="Internal").ap()
    g["table1"] = nc.dram_tensor("table1", [NC * SLOT1, T1COL], bf16,
                                 kind="Internal", addr_space="Shared").ap()
    g["hw2slots"] = nc.dram_tensor("hw2slots", [SLOT2, W2COL], bf16,
                                   kind="Internal").ap()
    g["table2"] = nc.dram_tensor("table2", [NC * SLOT2, W2COL], bf16,
                                 kind="Internal", addr_space="Shared").ap()

    with tile.TileContext(nc) as tc:
        _emit(nc, tc, J, g)
    nc.compile()
    return nc


def _emit(nc, tc, J, g):
    NCH = _nchunk()
    TCOLS = int(sum(J))
    with tc.tile_pool(name="res", bufs=1) as res:
        w1_sb = res.tile([P, 2, W1COL], bf16)
        nc.sync.dma_start(out=w1_sb[:],
                          in_=g["W1aug"][:, :].rearrange("(h p) c -> p h c",
                                                         h=2))
        w2_sb = res.tile([P, 2, W2COL], bf16)
        nc.sync.dma_start(out=w2_sb[:],
                          in_=g["W2aug"][:, :].rearrange("(h p) c -> p h c",
                                                         h=2))
        srcidx1_sb = res.tile([P, TCOLS], i32)
        nc.sync.dma_start(out=srcidx1_sb[:], in_=g["srcidx1"][:, :])
        srcidx2_sb = res.tile([P, TCOLS], i32)
        nc.sync.dma_start(out=srcidx2_sb[:], in_=g["srcidx2"][:, :])
        adidx_sb = res.tile([P, NCH], i32)
        nc.sync.dma_start(out=adidx_sb[:], in_=g["adidx"][:, :])

        ident_f = res.tile([P, P], f32)
        make_identity(nc, ident_f[:])
        ident = res.tile([P, P], bf16)
        nc.vector.tensor_copy(out=ident[:], in_=ident_f[:])
        ones_sb = res.tile([1, P], bf16)
        nc.vector.memset(ones_sb[:], 1.0)

        with tc.tile_pool(name="bb", bufs=1, space="PSUM") as bbp:
            b1_row = res.tile([1, D1], bf16)
            nc.sync.dma_start(out=b1_row[:], in_=g["b1"][:, :])
            b2_row = res.tile([1, NCLS], bf16)
            nc.sync.dma_start(out=b2_row[:], in_=g["b2"][:, :])
            one_row = res.tile([1, W2COL], bf16)
            nc.sync.dma_start(out=one_row[:], in_=g["onecol"][:, :])
            b1_ps = bbp.tile([P, D1], f32, space="PSUM", tag="b")
            nc.tensor.matmul(out=b1_ps[:], lhsT=ones_sb[:], rhs=b1_row[:],
                             start=True, stop=True)
            b1_bc = res.tile([P, D1], f32)
            nc.vector.tensor_copy(out=b1_bc[:], in_=b1_ps[:])
            b2_ps = bbp.tile([P, NCLS], f32, space="PSUM", tag="b")
            nc.tensor.matmul(out=b2_ps[:], lhsT=ones_sb[:], rhs=b2_row[:],
                             start=True, stop=True)
            b2_bc = res.tile([P, NCLS], f32)
            nc.vector.tensor_copy(out=b2_bc[:], in_=b2_ps[:])
            one_ps = bbp.tile([P, W2COL], f32, space="PSUM", tag="b")
            nc.tensor.matmul(out=one_ps[:], lhsT=ones_sb[:], rhs=one_row[:],
                             start=True, stop=True)
            one_bc = res.tile([P, W2COL], f32)
            nc.vector.tensor_copy(out=one_bc[:], in_=one_ps[:])

        # ---------------- Phase A (own shard)
        NBLK = (NSH + P - 1) // P
        with tc.tile_pool(name="pa_x", bufs=1) as pax, \
             tc.tile_pool(name="pa_o", bufs=3) as pao, \
             tc.tile_pool(name="pa_ps", bufs=2, space="PSUM") as paps:
            xts = pax.tile([P, 2, NSH], bf16)
            nc.sync.dma_start(
                out=xts[:],
                in_=g["xTs"][:, :].rearrange("(h p) n -> p h n", h=2))
            for j in range(NBLK):
                cw = min(P, NSH - j * P)
                ps = paps.tile([P, W1COL], f32, space="PSUM", tag="ps")
                for h in range(2):
                    nc.tensor.matmul(out=ps[:cw, :],
                                     lhsT=xts[:, h, j * P:j * P + cw],
                                     rhs=w1_sb[:, h, :],
                                     start=(h == 0), stop=(h == 1))
                o_t = pao.tile([P, W1COL], bf16, tag="o")
                if j % 2 == 0:
                    nc.scalar.copy(out=o_t[:cw, :], in_=ps[:cw, :])
                else:
                    nc.vector.tensor_copy(out=o_t[:cw, :], in_=ps[:cw, :])
                nc.sync.dma_start(out=g["t1shard"][j * P:j * P + cw, :],
                                  in_=o_t[:cw, :T1COL])
                nc.sync.dma_start(out=g["adshard"][j * P:j * P + cw, :],
                                  in_=o_t[:cw, T1COL:])
            sent_sb = pax.tile([1, T1COL], bf16)
            nc.sync.dma_start(out=sent_sb[:], in_=g["sent1"][:, :])
            nc.sync.dma_start(out=g["t1shard"][NSH:NSH + 1, :],
                              in_=sent_sb[:])

        nc.gpsimd.collective_compute(
            "AllGather", OP.bypass,
            replica_groups=[list(range(NC))],
            ins=[g["t1shard"][:, :].opt()], outs=[g["table1"][:, :].opt()])

        # pre-gather alpha_dst for every chunk (local adshard; overlaps AG)
        ads_all = res.tile([P, NCH, HEADS], bf16)
        for c in range(NCH):
            nc.gpsimd.indirect_dma_start(
                out=ads_all[:, c, :], out_offset=None, in_=g["adshard"][:, :],
                in_offset=bass.IndirectOffsetOnAxis(
                    ap=adidx_sb[:, c:c + 1], axis=0))

        _edge_layer(nc, tc, J, layer=1, table=g["table1"], row_w=T1COL,
                    nheads=HEADS, hid=HID, srcidx_sb=srcidx1_sb,
                    adidx_sb=adidx_sb, adshard=g["adshard"], b_bc=b1_bc,
                    one_bc=one_bc, ident=ident, w2_sb=w2_sb,
                    hw2slots=g["hw2slots"], out2=None, g=g,
                    ads_all=ads_all)

        nc.gpsimd.collective_compute(
            "AllGather", OP.bypass,
            replica_groups=[list(range(NC))],
            ins=[g["hw2slots"][:, :].opt()], outs=[g["table2"][:, :].opt()])

        _edge_layer(nc, tc, J, layer=2, table=g["table2"], row_w=W2COL,
                    nheads=1, hid=NCLS, srcidx_sb=srcidx2_sb,
                    adidx_sb=None, adshard=g["hw2slots"], b_bc=b2_bc,
                    one_bc=None, ident=ident, w2_sb=None, hw2slots=None,
                    out2=g["out2"], g=g, ads_all=None)


def _edge_layer(nc, tc, J, layer, table, row_w, nheads, hid, srcidx_sb,
                adidx_sb, adshard, b_bc, one_bc, ident, w2_sb, hw2slots,
                out2, g, ads_all):
    NCH = _nchunk()
    NH = nheads
    DW = NH * hid                 # 256 or 40
    GW = row_w                    # 264 or 44
    ACOL = DW if layer == 1 else NCLS
    SCW = T1COL if layer == 1 else W2COL
    SCP = 512 if layer == 1 else 128
    sfx = f"l{layer}"
    EB = 2 if layer == 1 else 4
    colbase = [0]
    for j in J:
        colbase.append(colbase[-1] + j)

    order = sorted(range(NCH), key=lambda c: J[c])   # ascending tile count

    with tc.tile_pool(name=f"g{sfx}", bufs=3) as gpool, \
         tc.tile_pool(name=f"m{sfx}", bufs=2) as mpool, \
         tc.tile_pool(name=f"e{sfx}", bufs=2) as epool, \
         tc.tile_pool(name=f"ac{sfx}", bufs=2, space="PSUM") as acp, \
         tc.tile_pool(name=f"tp{sfx}", bufs=1, space="PSUM") as tpp:
        acc = None
        eb0 = 0
        ebn = 0
        for pos, c in enumerate(order):
            Jc = J[c]
            G = gpool.tile([P, Jc, GW], bf16, tag="g")
            for t in range(Jc):
                cc = colbase[c] + t
                nc.gpsimd.indirect_dma_start(
                    out=G[:, t, :], out_offset=None, in_=table[:, :],
                    in_offset=bass.IndirectOffsetOnAxis(
                        ap=srcidx_sb[:, cc:cc + 1], axis=0))
            # alpha_dst for this chunk's 128 dsts
            if layer == 1:
                adt = ads_all[:, c, :]
            else:
                adt_t = mpool.tile([P, NH], bf16, tag="adt")
                nc.sync.dma_start(
                    out=adt_t[:],
                    in_=adshard[c * P:(c + 1) * P, NCLS + 1:NCLS + 2])
                adt = adt_t[:]

            # logit = alpha_src + alpha_dst; leaky_relu; exp (into G)
            lg = mpool.tile([P, Jc, NH], bf16, tag="lg")
            nc.vector.tensor_tensor(
                out=lg[:], in0=G[:, :, ACOL:ACOL + NH],
                in1=adt.unsqueeze(1).to_broadcast((P, Jc, NH)),
                op=OP.add)
            lr_n = mpool.tile([P, Jc, NH], bf16, tag="lrn")
            nc.vector.tensor_scalar(out=lr_n[:], in0=lg[:], scalar1=0.0,
                                    scalar2=NEG, op0=OP.min, op1=OP.mult)
            nc.vector.scalar_tensor_tensor(out=lg[:], in0=lg[:], scalar=0.0,
                                           in1=lr_n[:], op0=OP.max,
                                           op1=OP.add)
            nc.scalar.activation(out=G[:, :, ACOL:ACOL + NH], in_=lg[:],
                                 func=AF.Exp)
            if layer == 1:
                exe = mpool.tile([P, Jc, NH, hid], bf16, tag="exe")
                if c % 2 == 0:
                    nc.scalar.activation(
                        out=exe[:], in_=lg[:].unsqueeze(3)
                        .to_broadcast((P, Jc, NH, hid)), func=AF.Exp)
                else:
                    nc.vector.tensor_copy(
                        out=exe[:], in_=G[:, :, ACOL:ACOL + NH].unsqueeze(3)
                        .to_broadcast((P, Jc, NH, hid)))
                nc.vector.tensor_tensor(
                    out=G[:, :, :DW].rearrange("p t (h w) -> p t h w", h=NH),
                    in0=G[:, :, :DW].rearrange("p t (h w) -> p t h w", h=NH),
                    in1=exe[:], op=OP.mult)
            else:
                exe = mpool.tile([P, Jc, GW], bf16, tag="exe")
                nc.vector.tensor_copy(
                    out=exe[:], in_=G[:, :, ACOL:ACOL + 1]
                    .to_broadcast((P, Jc, GW)))
                nc.vector.tensor_tensor(out=G[:], in0=G[:], in1=exe[:],
                                        op=OP.mult)

            if pos % EB == 0:
                eb0 = pos
                ebn = min(EB, NCH - pos)
                acc = acp.tile([P, ebn, SCP], f32, space="PSUM", tag="acc")
            ei = pos - eb0
            for t in range(Jc):
                nc.tensor.matmul(out=acc[:, ei, :SCW], lhsT=ident[:],
                                 rhs=G[:, t, :SCW],
                                 start=(t == 0), stop=(t == Jc - 1))

            if ei != ebn - 1:
                continue

            # ---------------- epilogue for chunks order[eb0:eb0+ebn]
            batch = order[eb0:eb0 + ebn]
            EBv = ebn
            if layer == 2:
                den = epool.tile([P, EBv], f32, tag="den")
                nc.vector.tensor_scalar(out=den[:],
                                        in0=acc[:, :, NCLS + 2],
                                        scalar1=EPS, scalar2=None, op0=OP.max)
                rec = epool.tile([P, EBv], f32, tag="rec")
                nc.vector.reciprocal(out=rec[:], in_=den[:])
                o_sb = epool.tile([P, EBv, NCLS], f32, tag="osb")
                for e in range(EBv):
                    ce = batch[e]
                    nc.vector.scalar_tensor_tensor(
                        out=o_sb[:, e, :], in0=acc[:, e, :NCLS],
                        scalar=rec[:, e:e + 1], in1=b_bc[:],
                        op0=OP.mult, op1=OP.add)
                    nc.sync.dma_start(
                        out=out2[ce * P:(ce + 1) * P, :],
                        in_=o_sb[:, e, :])
                continue

            den = epool.tile([P, EBv, NH], f32, tag="den")
            nc.vector.tensor_scalar(out=den[:], in0=acc[:, :, DW:DW + NH],
                                    scalar1=EPS, scalar2=None, op0=OP.max)
            rec = epool.tile([P, EBv, NH], f32, tag="rec")
            nc.vector.reciprocal(out=rec[:], in_=den[:])
            outv = epool.tile([P, EBv, DW], f32, tag="outv")
            nc.vector.tensor_tensor(
                out=outv[:].rearrange("p e (h w) -> p e h w", h=NH),
                in0=acc[:, :, :DW].rearrange("p e (h w) -> p e h w", h=NH),
                in1=rec[:].unsqueeze(3).to_broadcast((P, EBv, NH, hid)),
                op=OP.mult)
            nc.vector.tensor_tensor(
                out=outv[:], in0=outv[:],
                in1=b_bc[:].unsqueeze(1).to_broadcast((P, EBv, DW)),
                op=OP.add)
            mneg = epool.tile([P, EBv, DW], f32, tag="mneg")
            nc.vector.tensor_scalar(out=mneg[:], in0=outv[:], scalar1=0.0,
                                    scalar2=None, op0=OP.min)
            expm = epool.tile([P, EBv, DW], f32, tag="expm")
            nc.scalar.activation(out=expm[:], in_=mneg[:], func=AF.Exp)
            rel1 = epool.tile([P, EBv, DW], f32, tag="rel1")
            nc.vector.tensor_scalar(out=rel1[:], in0=outv[:], scalar1=0.0,
                                    scalar2=1.0, op0=OP.max, op1=OP.subtract)
            h_sb = epool.tile([P, EBv, DW], bf16, tag="h")
            nc.vector.tensor_tensor(out=h_sb[:], in0=expm[:], in1=rel1[:],
                                    op=OP.add)

            for e in range(EBv):
                ce = batch[e]
                hT_sb = epool.tile([P, 2, P], bf16, tag="hT")
                for hh in range(2):
                    hT_ps = tpp.tile([P, P], bf16, space="PSUM", tag="hT")
                    nc.tensor.transpose(out=hT_ps[:],
                                        in_=h_sb[:, e, hh * P:(hh + 1) * P],
                                        identity=ident[:])
                    nc.vector.tensor_copy(out=hT_sb[:, hh, :], in_=hT_ps[:])
                hw_ps = tpp.tile([P, W2COL], f32, space="PSUM", tag="hw")
                for hh in range(2):
                    nc.tensor.matmul(out=hw_ps[:], lhsT=hT_sb[:, hh, :],
                                     rhs=w2_sb[:, hh, :],
                                     start=(hh == 0), stop=(hh == 1))
                hw_sb = epool.tile([P, W2COL], bf16, tag="hws")
                nc.vector.tensor_tensor(out=hw_sb[:], in0=hw_ps[:],
                                        in1=one_bc[:], op=OP.add)
                r0 = ce * P
                nc.sync.dma_start(out=hw2slots[r0:r0 + P, :], in_=hw_sb[:])
        if layer == 1:
            # sentinel row for table2
            with tc.tile_pool(name="s2", bufs=1) as s2p:
                sent_sb = s2p.tile([1, W2COL], bf16)
                nc.sync.dma_start(out=sent_sb[:], in_=g["sent2"][:, :])
                nc.sync.dma_start(out=hw2slots[NCH * P:NCH * P + 1, :],
                                  in_=sent_sb[:])


# ---------------------------------------------------------------- entry
def kernel(**inputs):
    global _last_bench
    args = {k: np.asarray(v) for k, v in inputs.items()}
    in_maps, J, perms = _prep(
        args["x"], args["edge_index"], args["W1"], args["a1_src"],
        args["a1_dst"], args["b1"], args["W2"], args["a2_src"],
        args["a2_dst"], args["b2"])
    if J not in _prog_cache:
        _prog_cache[J] = _build(J)
    nc = _prog_cache[J]
    trace = os.environ.get("GAT_TRACE", "0") == "1"
    r = bass_utils.run_bass_kernel_spmd(
        nc, in_maps, core_ids=list(range(NC)), trace=trace)
    _last_bench = r
    out = np.empty((N, NCLS), np.float32)
    for k in range(NC):
        o = r.results[k]["out2"]          # [NCH*P, 40] in rank order
        out[k * NSH:(k + 1) * NSH] = o[rankofinv(perms[k])]
    return out


def rankofinv(perm):
    """rows of out2 are rank-ordered; return index array mapping local dst
    id -> rank."""
    rank = np.empty(len(perm), np.int64)
    rank[perm] = np.arange(len(perm))
    return rank
